# revision 10
# baseline (speedup 1.0000x reference)
"""GNN message passing kernel for Trainium2 (8 NeuronCores).

out[v] = mean_{e: dst(e)=v} ( node_states[src(e)] @ a_in[e] )   [N=50000, D=32, E=400000]

Strategy (block-PSUM-accumulate):
  - Host: sort edges by dst, shard by dst range (no cross-core reduction).
    Partition each core's node range into variable-width node BLOCKS sized to
    <=128 nodes and <=1024 edges; each block's edges fill 8 chunks of 128 edge
    slots. Pre-gather source node states on host, pre-scaled by 1/indegree(dst)
    so the device segment-sum directly yields the mean. Per-edge matrices
    stored transposed as (k, d), everything partition-major in bf16.
  - Device (identical SPMD program, per-core data): per block:
      one DMA of the block's A (8 chunks); one DVE mult
      tmp[p,c,k,d] = A_T * x (bf16 2x); 8 tensor_scalar onehots
      oh[p,c,slot] = (slot_of_edge == iota) (bf16 4x);
      16 matmuls (N=512): psum[slot, (k,d)] += oh_c^T @ tmp_c accumulated
      over the block's 8 chunks (un-reduced products scattered to nodes);
      one DVE reduce over d of the aggregated psum -> [slot, k] (8x less
      reduce work than per-edge reduction since avg indegree ~8);
      DMA the block result out.
  - Host: blocks tile each core's node range contiguously; concatenate
    the first nodes_b rows of each block result.
"""

import sys

if "/opt/trn_rl_repo" not in sys.path:
    sys.path.insert(0, "/opt/trn_rl_repo")

import numpy as np

from concourse import bacc, bass, mybir, tile
from concourse.bass_utils import run_bass_kernel_spmd

P = 128
NCORES = 8
D = 32
CPB = 8  # chunks (of 128 edge slots) per node block

_PROGRAM_CACHE = {}


def _build_program(NBLK):
    """Per-core Bass program. NBLK node blocks, CPB chunks each."""
    fdt = mybir.dt.float32
    bdt = mybir.dt.bfloat16
    DD = D * D
    G = NBLK * CPB

    nc = bacc.Bacc(None, target_bir_lowering=False)

    a_dev = nc.declare_dram_parameter("a_dev", [P, G * DD], bdt, isOutput=False)
    x_dev = nc.declare_dram_parameter("x_dev", [P, G * D], bdt, isOutput=False)
    oh_dev = nc.declare_dram_parameter("oh_dev", [P, G * P], mybir.dt.float8e4, isOutput=False)
    out_d = nc.declare_dram_parameter("out", [P, NBLK * D], fdt, isOutput=True)

    with tile.TileContext(nc) as tc:
        with (
            tc.tile_pool(name="const", bufs=1) as cpool,
            tc.tile_pool(name="a", bufs=3) as apool,
            tc.tile_pool(name="tmp", bufs=3) as tpool,
            tc.tile_pool(name="oh", bufs=3) as opool,
            tc.tile_pool(name="red", bufs=3) as rpool,
            tc.tile_pool(name="ps", bufs=3, space="PSUM") as pspool,
        ):
            # ---- persistent tiles ----
            x_t = cpool.tile([P, G * D], bdt)
            nc.sync.dma_start(out=x_t[:], in_=x_dev[:])

            for b in range(NBLK):
                g0 = b * CPB
                a_t = apool.tile([P, CPB, DD], bdt)
                nc.sync.dma_start(
                    out=a_t[:],
                    in_=a_dev[:, g0 * DD : (g0 + CPB) * DD].rearrange(
                        "p (c f) -> p c f", f=DD
                    ),
                )
                # tmp[p, c, k, d] = A_T[p, c, k, d] * x[p, c, d]
                tmp_t = tpool.tile([P, CPB, D, D], bdt)
                nc.vector.tensor_tensor(
                    out=tmp_t[:],
                    in0=a_t[:].rearrange("p c (k d) -> p c k d", d=D),
                    in1=x_t[:, g0 * D : (g0 + CPB) * D]
                    .rearrange("p (c d) -> p c d", d=D)
                    .unsqueeze(2)
                    .to_broadcast([P, CPB, D, D]),
                    op=mybir.AluOpType.mult,
                )
                # host-precomputed onehots (0/1 exact in fp8)
                oh_t = opool.tile([P, CPB, P], mybir.dt.float8e4)
                nc.sync.dma_start(
                    out=oh_t[:],
                    in_=oh_dev[:, g0 * P : (g0 + CPB) * P].rearrange(
                        "p (c s) -> p c s", s=P
                    ),
                )
                # psum[slot, (k,d)] += oh_c^T @ tmp_c over the block's chunks
                ps_t = pspool.tile([P, DD], fdt, space="PSUM")
                for c in range(CPB):
                    for h in range(2):
                        nc.tensor.matmul(
                            out=ps_t[:, h * 512 : (h + 1) * 512],
                            lhsT=oh_t[:, c, :],
                            rhs=tmp_t[:, c, :, :].rearrange("p k d -> p (k d)")[
                                :, h * 512 : (h + 1) * 512
                            ],
                            start=(c == 0),
                            stop=(c == CPB - 1),
                        )
                # reduce over d: [slot, k]
                red_t = rpool.tile([P, D], fdt)
                nc.vector.tensor_reduce(
                    out=red_t[:],
                    in_=ps_t[:].rearrange("p (k d) -> p k d", d=D),
                    axis=mybir.AxisListType.X,
                    op=mybir.AluOpType.add,
                )
                nc.sync.dma_start(
                    out=out_d[:, b * D : (b + 1) * D], in_=red_t[:]
                )

    nc.compile()
    return nc


def _blocks_core(dst_l, nb):
    """Partition local node range [0, nb) into blocks of <=128 nodes and
    <=CPB*128 edges. Returns (n0, n_nodes, e0, n_edges) per block."""
    deg = np.bincount(dst_l, minlength=nb)
    cum = np.concatenate(([0], np.cumsum(deg)))
    emax = CPB * P
    blocks = []
    n0 = 0
    while n0 < nb:
        hi = int(np.searchsorted(cum, cum[n0] + emax, side="right")) - 1
        nn = min(hi - n0, P)
        assert nn >= 1, f"node degree {deg[n0]} exceeds {emax}"
        blocks.append((n0, nn, int(cum[n0]), int(cum[n0 + nn] - cum[n0])))
        n0 += nn
    return blocks


def prepare(node_states, a_in, edge_index):
    """Host-side sharding/packing. Returns (nc, in_maps, unpack, nb, N, NBLK)."""
    node_states = np.asarray(node_states, dtype=np.float32)
    a_in = np.asarray(a_in, dtype=np.float32)
    edge_index = np.asarray(edge_index)

    N, Dn = node_states.shape
    assert Dn == D
    DD = D * D

    nb = (N + NCORES - 1) // NCORES

    src = edge_index[:, 0].astype(np.int64)
    dst = edge_index[:, 1].astype(np.int64)

    # per-node 1/indegree, folded into the gathered source states
    cnt = np.bincount(dst, minlength=N).astype(np.float32)
    inv_cnt = 1.0 / np.maximum(cnt, 1.0)

    order = np.argsort(dst, kind="stable")
    dst_s = dst[order]
    cuts = np.searchsorted(dst_s, [c * nb for c in range(NCORES + 1)])

    core_blocks = []
    for c in range(NCORES):
        lo, hi = cuts[c], cuts[c + 1]
        nb_c = min(nb, N - c * nb)
        core_blocks.append((lo, hi, _blocks_core(dst_s[lo:hi] - c * nb, nb_c)))

    NBLK = max(len(b) for _, _, b in core_blocks)
    G = NBLK * CPB

    import ml_dtypes

    bdt_np = np.dtype(ml_dtypes.bfloat16)


    in_maps = []
    unpack = []  # per core: row_ids into [NBLK*P] block-slot space
    for c in range(NCORES):
        lo, hi, blocks = core_blocks[c]
        Ec = hi - lo
        eg = order[lo:hi]
        nblk_c = len(blocks)

        n0_arr = np.array([b[0] for b in blocks], dtype=np.int64)
        nn_arr = np.array([b[1] for b in blocks], dtype=np.int64)
        eb_arr = np.array([b[3] for b in blocks], dtype=np.int64)

        # per-edge coordinates (edges sorted by dst tile the blocks in order)
        e_blk = np.repeat(np.arange(nblk_c), eb_arr)
        pos = np.arange(Ec, dtype=np.int64) - np.repeat(
            np.array([b[2] for b in blocks], dtype=np.int64), eb_arr
        )
        g_arr = e_blk * CPB + pos // P
        p_arr = pos % P
        slot_arr = dst_s[lo:hi] - c * nb - np.repeat(n0_arr, eb_arr)

        # A rows: gather + transpose to (k, d), cast bf16, scatter into
        # partition-major [P, G, D*D]
        a_c = np.zeros((P, G, DD), dtype=bdt_np)
        blk = a_in[eg].transpose(0, 2, 1).reshape(Ec, DD).astype(bdt_np)
        a_c[p_arr, g_arr, :] = blk

        # source node states, gathered on host, pre-scaled by 1/indegree(dst)
        x_c = np.zeros((P, G, D), dtype=bdt_np)
        xg = node_states[src[eg]] * inv_cnt[dst_s[lo:hi]][:, None]
        x_c[p_arr, g_arr, :] = xg.astype(bdt_np)

        f8_np = np.dtype(ml_dtypes.float8_e4m3)
        oh_c = np.zeros((P, G, P), dtype=f8_np)
        oh_c[p_arr, g_arr, slot_arr] = 1.0

        # unpack map: block b contributes rows b*P .. b*P+nn_b-1
        row_ids = np.concatenate(
            [b * P + np.arange(nn_arr[b]) for b in range(nblk_c)]
        ) if nblk_c else np.zeros(0, np.int64)
        unpack.append(row_ids)

        in_maps.append(
            {
                "a_dev": a_c.reshape(P, G * DD),
                "x_dev": x_c.reshape(P, G * D),
                "oh_dev": oh_c.reshape(P, G * P),
            }
        )

    if NBLK not in _PROGRAM_CACHE:
        _PROGRAM_CACHE[NBLK] = _build_program(NBLK)
    nc = _PROGRAM_CACHE[NBLK]
    return nc, in_maps, unpack, nb, N, NBLK


def kernel(node_states, a_in, edge_index):
    nc, in_maps, unpack, nb, N, NBLK = prepare(node_states, a_in, edge_index)
    global LAST_RESULT
    res = run_bass_kernel_spmd(nc, in_maps, list(range(NCORES)), trace=TRACE)
    LAST_RESULT = res
    out = np.zeros((NCORES * nb, D), dtype=np.float32)
    for c in range(NCORES):
        row_ids = unpack[c]
        rows = res.results[c]["out"].reshape(P, NBLK, D).transpose(1, 0, 2)
        out[c * nb : c * nb + len(row_ids)] = rows.reshape(NBLK * P, D)[row_ids]
    return out[:N]


TRACE = False
LAST_RESULT = None

if __name__ == "__main__":
    rng = np.random.default_rng(0)
    Nt, Et = 1024, 4096
    ns = rng.standard_normal((Nt, D)).astype(np.float32)
    ai = rng.standard_normal((Et, D, D)).astype(np.float32)
    ei = np.stack(
        [rng.integers(0, Nt, Et), rng.integers(0, Nt, Et)], axis=1
    ).astype(np.int64)
    got = kernel(ns, ai, ei)
    msg = np.einsum("ed,edk->ek", ns[ei[:, 0]], ai)
    sums = np.zeros((Nt, D), dtype=np.float32)
    np.add.at(sums, ei[:, 1], msg)
    cnt = np.zeros((Nt,), dtype=np.float32)
    np.add.at(cnt, ei[:, 1], 1.0)
    exp = sums / np.maximum(cnt, 1.0)[:, None]
    err = np.abs(got - exp).max() / (np.abs(exp).max() + 1e-9)
    print("max-abs-rel err:", err)


# revision 11
# speedup vs baseline: 1.1449x; 1.1449x over previous
"""GNN message passing kernel for Trainium2 (8 NeuronCores).

out[v] = mean_{e: dst(e)=v} ( node_states[src(e)] @ a_in[e] )   [N=50000, D=32, E=400000]

Strategy (block-PSUM-accumulate):
  - Host: sort edges by dst, shard by dst range (no cross-core reduction).
    Partition each core's node range into variable-width node BLOCKS sized to
    <=128 nodes and <=1024 edges; each block's edges fill 8 chunks of 128 edge
    slots. Pre-gather source node states on host, pre-scaled by 1/indegree(dst)
    so the device segment-sum directly yields the mean. Per-edge matrices
    stored transposed as (k, d), everything partition-major in bf16.
  - Device (identical SPMD program, per-core data): per block:
      one DMA of the block's A (8 chunks); one DVE mult
      tmp[p,c,k,d] = A_T * x (bf16 2x); 8 tensor_scalar onehots
      oh[p,c,slot] = (slot_of_edge == iota) (bf16 4x);
      16 matmuls (N=512): psum[slot, (k,d)] += oh_c^T @ tmp_c accumulated
      over the block's 8 chunks (un-reduced products scattered to nodes);
      one DVE reduce over d of the aggregated psum -> [slot, k] (8x less
      reduce work than per-edge reduction since avg indegree ~8);
      DMA the block result out.
  - Host: blocks tile each core's node range contiguously; concatenate
    the first nodes_b rows of each block result.
"""

import sys

if "/opt/trn_rl_repo" not in sys.path:
    sys.path.insert(0, "/opt/trn_rl_repo")

import numpy as np

from concourse import bacc, bass, mybir, tile
from concourse.bass_utils import run_bass_kernel_spmd

P = 128
NCORES = 8
D = 32
CPB = 8  # chunks (of 128 edge slots) per node block
OB = 4  # blocks per output-stage DMA

_PROGRAM_CACHE = {}


def _build_program(NBLK):
    """Per-core Bass program. NBLK node blocks, CPB chunks each."""
    fdt = mybir.dt.float32
    bdt = mybir.dt.bfloat16
    DD = D * D
    G = NBLK * CPB

    nc = bacc.Bacc(None, target_bir_lowering=False)

    a_dev = nc.declare_dram_parameter("a_dev", [P, G * DD], bdt, isOutput=False)
    x_dev = nc.declare_dram_parameter("x_dev", [P, G * D], bdt, isOutput=False)
    sloteq_d = nc.declare_dram_parameter("sloteq", [P, G], fdt, isOutput=False)
    iota_d = nc.declare_dram_parameter("iota", [P, P], fdt, isOutput=False)
    out_d = nc.declare_dram_parameter("out", [P, NBLK * D], fdt, isOutput=True)

    with tile.TileContext(nc) as tc:
        with (
            tc.tile_pool(name="const", bufs=1) as cpool,
            tc.tile_pool(name="a", bufs=4) as apool,
            tc.tile_pool(name="tmp", bufs=3) as tpool,
            tc.tile_pool(name="oh", bufs=3) as opool,
            tc.tile_pool(name="red", bufs=3) as rpool,
            tc.tile_pool(name="ps", bufs=3, space="PSUM") as pspool,
        ):
            # ---- persistent tiles ----
            iota_t = cpool.tile([P, P], fdt)
            nc.sync.dma_start(out=iota_t[:], in_=iota_d[:])
            sloteq_t = cpool.tile([P, G], fdt)
            nc.sync.dma_start(out=sloteq_t[:], in_=sloteq_d[:])
            x_t = cpool.tile([P, G * D], bdt)
            XSPLIT = min(8 * CPB * D, G * D)
            nc.sync.dma_start(out=x_t[:, 0:XSPLIT], in_=x_dev[:, 0:XSPLIT])
            if XSPLIT < G * D:
                nc.sync.dma_start(
                    out=x_t[:, XSPLIT : G * D], in_=x_dev[:, XSPLIT : G * D]
                )

            for b in range(NBLK):
                g0 = b * CPB
                a_t = apool.tile([P, CPB, DD], bdt)
                nc.sync.dma_start(
                    out=a_t[:],
                    in_=a_dev[:, g0 * DD : (g0 + CPB) * DD].rearrange(
                        "p (c f) -> p c f", f=DD
                    ),
                )
                # tmp[p, c, k, d] = A_T[p, c, k, d] * x[p, c, d]
                tmp_t = tpool.tile([P, CPB, D, D], bdt)
                nc.vector.tensor_tensor(
                    out=tmp_t[:],
                    in0=a_t[:].rearrange("p c (k d) -> p c k d", d=D),
                    in1=x_t[:, g0 * D : (g0 + CPB) * D]
                    .rearrange("p (c d) -> p c d", d=D)
                    .unsqueeze(2)
                    .to_broadcast([P, CPB, D, D]),
                    op=mybir.AluOpType.mult,
                )
                # oh[p, c, s] = (slot[p, c] == s), one batched compare per block
                oh_t = opool.tile([P, CPB, P], bdt)
                nc.vector.tensor_tensor(
                    out=oh_t[:],
                    in0=iota_t[:].unsqueeze(1).to_broadcast([P, CPB, P]),
                    in1=sloteq_t[:, g0 : g0 + CPB]
                    .unsqueeze(2)
                    .to_broadcast([P, CPB, P]),
                    op=mybir.AluOpType.is_equal,
                )
                # psum[slot, (k,d)] += oh_c^T @ tmp_c over the block's chunks
                ps_t = pspool.tile([P, DD], fdt, space="PSUM")
                for c in range(CPB):
                    for h in range(2):
                        nc.tensor.matmul(
                            out=ps_t[:, h * 512 : (h + 1) * 512],
                            lhsT=oh_t[:, c, :],
                            rhs=tmp_t[:, c, :, :].rearrange("p k d -> p (k d)")[
                                :, h * 512 : (h + 1) * 512
                            ],
                            start=(c == 0),
                            stop=(c == CPB - 1),
                        )
                # reduce over d: [slot, k], staged and flushed every OB blocks
                if b % OB == 0:
                    ob0 = b
                    obn = min(OB, NBLK - b)
                    stage_t = rpool.tile([P, obn * D], fdt)
                nc.vector.tensor_reduce(
                    out=stage_t[:, (b - ob0) * D : (b - ob0 + 1) * D],
                    in_=ps_t[:].rearrange("p (k d) -> p k d", d=D),
                    axis=mybir.AxisListType.X,
                    op=mybir.AluOpType.add,
                )
                if b - ob0 == obn - 1:
                    nc.sync.dma_start(
                        out=out_d[:, ob0 * D : (ob0 + obn) * D], in_=stage_t[:]
                    )

    nc.compile()
    return nc


def _blocks_core(dst_l, nb):
    """Partition local node range [0, nb) into blocks of <=128 nodes and
    <=CPB*128 edges. Returns (n0, n_nodes, e0, n_edges) per block."""
    deg = np.bincount(dst_l, minlength=nb)
    cum = np.concatenate(([0], np.cumsum(deg)))
    emax = CPB * P
    blocks = []
    n0 = 0
    while n0 < nb:
        hi = int(np.searchsorted(cum, cum[n0] + emax, side="right")) - 1
        nn = min(hi - n0, P)
        assert nn >= 1, f"node degree {deg[n0]} exceeds {emax}"
        blocks.append((n0, nn, int(cum[n0]), int(cum[n0 + nn] - cum[n0])))
        n0 += nn
    return blocks


def prepare(node_states, a_in, edge_index):
    """Host-side sharding/packing. Returns (nc, in_maps, unpack, nb, N, NBLK)."""
    node_states = np.asarray(node_states, dtype=np.float32)
    a_in = np.asarray(a_in, dtype=np.float32)
    edge_index = np.asarray(edge_index)

    N, Dn = node_states.shape
    assert Dn == D
    DD = D * D

    nb = (N + NCORES - 1) // NCORES

    src = edge_index[:, 0].astype(np.int64)
    dst = edge_index[:, 1].astype(np.int64)

    # per-node 1/indegree, folded into the gathered source states
    cnt = np.bincount(dst, minlength=N).astype(np.float32)
    inv_cnt = 1.0 / np.maximum(cnt, 1.0)

    order = np.argsort(dst, kind="stable")
    dst_s = dst[order]
    cuts = np.searchsorted(dst_s, [c * nb for c in range(NCORES + 1)])

    core_blocks = []
    for c in range(NCORES):
        lo, hi = cuts[c], cuts[c + 1]
        nb_c = min(nb, N - c * nb)
        core_blocks.append((lo, hi, _blocks_core(dst_s[lo:hi] - c * nb, nb_c)))

    NBLK = max(len(b) for _, _, b in core_blocks)
    G = NBLK * CPB

    import ml_dtypes

    bdt_np = np.dtype(ml_dtypes.bfloat16)

    iota_np = np.broadcast_to(
        np.arange(P, dtype=np.float32)[None, :], (P, P)
    ).copy()

    in_maps = []
    unpack = []  # per core: row_ids into [NBLK*P] block-slot space
    for c in range(NCORES):
        lo, hi, blocks = core_blocks[c]
        Ec = hi - lo
        eg = order[lo:hi]
        nblk_c = len(blocks)

        n0_arr = np.array([b[0] for b in blocks], dtype=np.int64)
        nn_arr = np.array([b[1] for b in blocks], dtype=np.int64)
        eb_arr = np.array([b[3] for b in blocks], dtype=np.int64)

        # per-edge coordinates (edges sorted by dst tile the blocks in order)
        e_blk = np.repeat(np.arange(nblk_c), eb_arr)
        pos = np.arange(Ec, dtype=np.int64) - np.repeat(
            np.array([b[2] for b in blocks], dtype=np.int64), eb_arr
        )
        g_arr = e_blk * CPB + pos // P
        p_arr = pos % P
        slot_arr = dst_s[lo:hi] - c * nb - np.repeat(n0_arr, eb_arr)

        # A rows: gather + transpose to (k, d), cast bf16, scatter into
        # partition-major [P, G, D*D]
        a_c = np.zeros((P, G, DD), dtype=bdt_np)
        blk = a_in[eg].transpose(0, 2, 1).reshape(Ec, DD).astype(bdt_np)
        a_c[p_arr, g_arr, :] = blk

        # source node states, gathered on host, pre-scaled by 1/indegree(dst)
        x_c = np.zeros((P, G, D), dtype=bdt_np)
        xg = node_states[src[eg]] * inv_cnt[dst_s[lo:hi]][:, None]
        x_c[p_arr, g_arr, :] = xg.astype(bdt_np)

        sloteq_c = np.full((P, G), -1.0, dtype=np.float32)
        sloteq_c[p_arr, g_arr] = slot_arr.astype(np.float32)

        # unpack map: block b contributes rows b*P .. b*P+nn_b-1
        row_ids = np.concatenate(
            [b * P + np.arange(nn_arr[b]) for b in range(nblk_c)]
        ) if nblk_c else np.zeros(0, np.int64)
        unpack.append(row_ids)

        in_maps.append(
            {
                "a_dev": a_c.reshape(P, G * DD),
                "x_dev": x_c.reshape(P, G * D),
                "sloteq": sloteq_c,
                "iota": iota_np,
            }
        )

    if NBLK not in _PROGRAM_CACHE:
        _PROGRAM_CACHE[NBLK] = _build_program(NBLK)
    nc = _PROGRAM_CACHE[NBLK]
    return nc, in_maps, unpack, nb, N, NBLK


def kernel(node_states, a_in, edge_index):
    nc, in_maps, unpack, nb, N, NBLK = prepare(node_states, a_in, edge_index)
    global LAST_RESULT
    res = run_bass_kernel_spmd(nc, in_maps, list(range(NCORES)), trace=TRACE)
    LAST_RESULT = res
    out = np.zeros((NCORES * nb, D), dtype=np.float32)
    for c in range(NCORES):
        row_ids = unpack[c]
        rows = res.results[c]["out"].reshape(P, NBLK, D).transpose(1, 0, 2)
        out[c * nb : c * nb + len(row_ids)] = rows.reshape(NBLK * P, D)[row_ids]
    return out[:N]


TRACE = False
LAST_RESULT = None

if __name__ == "__main__":
    rng = np.random.default_rng(0)
    Nt, Et = 1024, 4096
    ns = rng.standard_normal((Nt, D)).astype(np.float32)
    ai = rng.standard_normal((Et, D, D)).astype(np.float32)
    ei = np.stack(
        [rng.integers(0, Nt, Et), rng.integers(0, Nt, Et)], axis=1
    ).astype(np.int64)
    got = kernel(ns, ai, ei)
    msg = np.einsum("ed,edk->ek", ns[ei[:, 0]], ai)
    sums = np.zeros((Nt, D), dtype=np.float32)
    np.add.at(sums, ei[:, 1], msg)
    cnt = np.zeros((Nt,), dtype=np.float32)
    np.add.at(cnt, ei[:, 1], 1.0)
    exp = sums / np.maximum(cnt, 1.0)[:, None]
    err = np.abs(got - exp).max() / (np.abs(exp).max() + 1e-9)
    print("max-abs-rel err:", err)
